# revision 1
# baseline (speedup 1.0000x reference)
"""LIF spike kernel (T=4 scan with threshold reset) on 8 TRN2 NeuronCores.

Recurrence per element (tau=1, thresh=1):
    s_t     = m_{t-1} + x_t
    spike_t = (s_t > 1)           -> output, f32 0/1
    m_t     = s_t * (s_t <= 1)    -> threshold reset

Sharding: pure data-parallel over the batch axis (dim 1, 64 -> 8 per core).
Each core streams its [4, 1048576] f32 slice through SBUF in [128, F]
chunks, runs the 4-step scan on the Vector engine, and streams spikes out.

DMA: per chunk, all 4 timesteps are moved by ONE strided dma_start
([128, 4*F] SBUF tile; DRAM pattern is 4 runs of F*4 bytes at stride N*4).
Loads issue on the SP HWDGE ring, stores on the ACT HWDGE ring so a store
waiting on compute never head-of-line blocks the next chunk's load.
"""

import numpy as np

import concourse.bacc as bacc
import concourse.mybir as mybir
import concourse.tile as tile
from concourse import bass_utils

T = 4
B_FULL = 64
C, H, W = 128, 32, 32
N_CORES = 8
B_LOC = B_FULL // N_CORES            # 8
N = B_LOC * C * H * W                # 1048576 elements per core per timestep
P = 128                              # SBUF partitions

_GT = mybir.AluOpType.is_gt
_LE = mybir.AluOpType.is_le
_MUL = mybir.AluOpType.mult
_ADD = mybir.AluOpType.add

_nc_cache = None


def _build(F=1024, bufs=2, split_store_ring=True, coalesce_t=False, repeat=1):
    nchunk = N // (P * F)
    nc = bacc.Bacc(
        "TRN2",
        target_bir_lowering=False,
        debug=False,
        enable_asserts=False,
    )
    x_d = nc.dram_tensor("x", [T, N], mybir.dt.float32, kind="ExternalInput").ap()
    y_d = nc.dram_tensor("y", [T, N], mybir.dt.float32, kind="ExternalOutput").ap()
    # [t, n, p, f] view of the flat [T, N] DRAM tensors
    xv = x_d.rearrange("t (n p f) -> t n p f", p=P, f=F)
    yv = y_d.rearrange("t (n p f) -> t n p f", p=P, f=F)
    # [n, p, t, f] view: per (chunk, partition) the 4 timesteps' rows
    xc = x_d.rearrange("t (n p f) -> n p t f", p=P, f=F)
    yc = y_d.rearrange("t (n p f) -> n p t f", p=P, f=F)

    store_eng_of = (lambda _: nc.scalar) if split_store_ring else (lambda _: nc.sync)

    with tile.TileContext(nc) as tc:
        with (
            tc.tile_pool(name="xin", bufs=bufs) as xp,
            tc.tile_pool(name="spk", bufs=bufs) as spp,
            tc.tile_pool(name="wrk", bufs=bufs) as wkp,
        ):
            for j in range(nchunk * repeat):
                j = j % nchunk
                if coalesce_t:
                    xall = xp.tile(
                        [P, T * F], mybir.dt.float32, tag="x", name=f"x_{j}"
                    )
                    nc.sync.dma_start(
                        xall[:].rearrange("p (t f) -> p t f", t=T), xc[j]
                    )
                    xt = [xall[:, t * F : (t + 1) * F] for t in range(T)]
                    spall = spp.tile(
                        [P, T * F], mybir.dt.float32, tag="s", name=f"s_{j}"
                    )
                    sp = [spall[:, t * F : (t + 1) * F] for t in range(T)]
                else:
                    xt = []
                    for t in range(T):
                        xtile = xp.tile(
                            [P, F], mybir.dt.float32, tag=f"x{t}", name=f"x{t}_{j}"
                        )
                        nc.sync.dma_start(xtile[:], xv[t, j])
                        xt.append(xtile[:])
                    sp = []
                    for t in range(T):
                        stile = spp.tile(
                            [P, F], mybir.dt.float32, tag=f"s{t}", name=f"s{t}_{j}"
                        )
                        sp.append(stile[:])
                m = wkp.tile([P, F], mybir.dt.float32, tag="m", name=f"m_{j}")

                v = nc.vector
                # t = 0: m_prev = 0, so s = x0 directly
                v.tensor_single_scalar(sp[0], xt[0], 1.0, _GT)
                v.scalar_tensor_tensor(m[:], xt[0], 1.0, xt[0], _LE, _MUL)
                for t in range(1, T):
                    v.tensor_tensor(m[:], m[:], xt[t], _ADD)
                    v.tensor_single_scalar(sp[t], m[:], 1.0, _GT)
                    if t < T - 1:  # m after the last step is dead
                        v.scalar_tensor_tensor(m[:], m[:], 1.0, m[:], _LE, _MUL)

                if coalesce_t:
                    store_eng_of(j).dma_start(
                        yc[j], spall[:].rearrange("p (t f) -> p t f", t=T)
                    )
                else:
                    for t in range(T):
                        store_eng_of(j).dma_start(yv[t, j], sp[t])

    nc.compile()
    return nc


def _build_perm(F=2048, bufs=2, repeat=1):
    """Host-permuted layout: DRAM is [nchunk, P, T*F] so each chunk moves as
    ONE contiguous T*F*P*4-byte DMA each way. Spikes are written in place
    over the x tile (x[t] is dead after the add), halving SBUF."""
    nchunk = N // (P * F)
    nc = bacc.Bacc(
        "TRN2",
        target_bir_lowering=False,
        debug=False,
        enable_asserts=False,
    )
    x_d = nc.dram_tensor("x", [T * N], mybir.dt.float32, kind="ExternalInput").ap()
    y_d = nc.dram_tensor("y", [T * N], mybir.dt.float32, kind="ExternalOutput").ap()
    xv = x_d.rearrange("(n p q) -> n p q", p=P, q=T * F)
    yv = y_d.rearrange("(n p q) -> n p q", p=P, q=T * F)

    with tile.TileContext(nc) as tc:
        with (
            tc.tile_pool(name="io", bufs=bufs) as iop,
            tc.tile_pool(name="wrk", bufs=bufs) as wkp,
        ):
            for j in range(nchunk * repeat):
                j = j % nchunk
                xall = iop.tile([P, T * F], mybir.dt.float32, tag="x", name=f"x_{j}")
                nc.sync.dma_start(xall[:], xv[j])
                sl = [xall[:, t * F : (t + 1) * F] for t in range(T)]
                m = wkp.tile([P, F], mybir.dt.float32, tag="m", name=f"m_{j}")

                v = nc.vector
                # t = 0: m init from x0 first, then spike0 overwrites x0
                v.scalar_tensor_tensor(m[:], sl[0], 1.0, sl[0], _LE, _MUL)
                v.tensor_single_scalar(sl[0], sl[0], 1.0, _GT)
                for t in range(1, T):
                    v.tensor_tensor(m[:], m[:], sl[t], _ADD)
                    v.tensor_single_scalar(sl[t], m[:], 1.0, _GT)
                    if t < T - 1:
                        v.scalar_tensor_tensor(m[:], m[:], 1.0, m[:], _LE, _MUL)

                nc.scalar.dma_start(yv[j], xall[:])

    nc.compile()
    return nc


_PERM = False
_PERM_F = 2048


def _get_nc():
    global _nc_cache
    if _nc_cache is None:
        _nc_cache = _build_perm(F=_PERM_F) if _PERM else _build()
    return _nc_cache


def _permute_in(x_core, F):
    """[T, N] -> flat [T*N] in (nchunk, P, T, F) order."""
    nchunk = N // (P * F)
    return np.ascontiguousarray(
        x_core.reshape(T, nchunk, P, F).transpose(1, 2, 0, 3)
    ).reshape(T * N)


def _unpermute_out(y_flat, F):
    nchunk = N // (P * F)
    return np.ascontiguousarray(
        y_flat.reshape(nchunk, P, T, F).transpose(2, 0, 1, 3)
    ).reshape(T, N)


def _run(x, **spmd_kwargs):
    x = np.asarray(x, dtype=np.float32)
    assert x.shape == (T, B_FULL, C, H, W), x.shape
    if _PERM:
        in_maps = [
            {
                "x": _permute_in(
                    np.ascontiguousarray(
                        x[:, c * B_LOC : (c + 1) * B_LOC]
                    ).reshape(T, N),
                    _PERM_F,
                )
            }
            for c in range(N_CORES)
        ]
    else:
        in_maps = [
            {
                "x": np.ascontiguousarray(
                    x[:, c * B_LOC : (c + 1) * B_LOC]
                ).reshape(T, N)
            }
            for c in range(N_CORES)
        ]
    res = bass_utils.run_bass_kernel_spmd(
        _get_nc(), in_maps, core_ids=list(range(N_CORES)), **spmd_kwargs
    )
    out = np.empty((T, B_FULL, C, H, W), dtype=np.float32)
    for c in range(N_CORES):
        y = res.results[c]["y"]
        if _PERM:
            y = _unpermute_out(y, _PERM_F)
        out[:, c * B_LOC : (c + 1) * B_LOC] = y.reshape(T, B_LOC, C, H, W)
    return out, res


def kernel(x):
    out, _ = _run(x)
    return out

